# revision 4
# baseline (speedup 1.0000x reference)
"""InterAttention kernel for Trainium2 (8 NeuronCores, data-parallel over batch).

Per batch b:
  q_proj = q_encode @ G_weight.T + G_bias            [LQ, H]
  att    = p_encode @ q_proj.T                       [LP, LQ]
  m_p    = softmax(att, axis=-1) @ q_encode          [LP, H]
  m_q    = softmax(att, axis=-2).T @ p_encode        [LQ, H]

B=32, LP=2048, LQ=512, H=1024.  8 cores x 4 batches each.
fp32 matmul chain through att (softmax-logit accuracy), bf16 post-softmax.
"""

import sys
import types

import numpy as np

# ---------------------------------------------------------------------------
# NTFF profile hook injection (lets run_bass_kernel_spmd(trace=True) work
# under axon). Harmless if the boot module is missing.
_HOOK = [None]


def _install_ntff_hook():
    if "antenv.axon_hooks" not in sys.modules:
        mod = types.ModuleType("antenv.axon_hooks")
        mod.set_axon_ntff_profile_hook = lambda h: _HOOK.__setitem__(0, h)
        mod.get_axon_ntff_profile_hook = lambda: _HOOK[0]
        sys.modules["antenv.axon_hooks"] = mod
        try:
            import antenv

            antenv.axon_hooks = mod
        except ImportError:
            pass
    mod = sys.modules["antenv.axon_hooks"]
    if mod.get_axon_ntff_profile_hook() is None:
        try:
            from trn_agent_boot.trn_boot import _ntff_profile_via_ctypes

            mod.set_axon_ntff_profile_hook(
                _ntff_profile_via_ctypes("/opt/axon/libaxon_pjrt.so")
            )
        except Exception:
            pass


_install_ntff_hook()

import concourse.bass as bass  # noqa: E402
import concourse.mybir as mybir  # noqa: E402
from concourse import bacc  # noqa: E402
from concourse.bass_utils import run_bass_kernel_spmd  # noqa: E402
from concourse.masks import make_identity  # noqa: E402
from concourse.tile import TileContext  # noqa: E402

P = 128
B, LP, LQ, H = 32, 2048, 512, 1024
N_CORES = 8
BPC = B // N_CORES  # batches per core
PT = LP // P  # 16 p-tiles
QT = LQ // P  # 4 q-tiles
HT = H // P  # 8 h-tiles
F32 = mybir.dt.float32
BF16 = mybir.dt.bfloat16
EXP = mybir.ActivationFunctionType.Exp
ADD = mybir.AluOpType.add
MAX = mybir.AluOpType.max
AX = mybir.AxisListType.X


def build_nc(trace_scopes=False):
    nc = bacc.Bacc("TRN2", target_bir_lowering=False, debug=False)

    pe_d = nc.dram_tensor("p_encode", [BPC, LP, H], F32, kind="ExternalInput")
    qe_d = nc.dram_tensor("q_encode", [BPC, LQ, H], F32, kind="ExternalInput")
    gw_d = nc.dram_tensor("G_weight", [H, H], F32, kind="ExternalInput")
    gb_d = nc.dram_tensor("G_bias", [H], F32, kind="ExternalInput")
    mp_d = nc.dram_tensor("m_p", [BPC, LP, H], F32, kind="ExternalOutput")
    mq_d = nc.dram_tensor("m_q", [BPC, LQ, H], F32, kind="ExternalOutput")

    pe_a = pe_d.ap().rearrange("b (t p) h -> b t p h", p=P)  # [BPC,16,128,1024]
    qe_a = qe_d.ap().rearrange("b (t p) h -> b t p h", p=P)  # [BPC,4,128,1024]
    gw_a = gw_d.ap().rearrange("(t p) h -> t p h", p=P)  # [8,128,1024]
    mp_a = mp_d.ap().rearrange("b (t p) h -> b t p h", p=P)
    mq_a = mq_d.ap().rearrange("b (t p) h -> b t p h", p=P)

    with TileContext(nc) as tc:
        with (
            tc.tile_pool(name="consts", bufs=1) as consts,
            tc.tile_pool(name="glob", bufs=1) as glob,
            tc.tile_pool(name="per_batch", bufs=1) as pb,
            tc.tile_pool(name="stream", bufs=3) as st,
            tc.tile_pool(name="small", bufs=2) as sm,
            tc.tile_pool(name="outs", bufs=2) as outp,
            tc.tile_pool(name="psum", bufs=1, space="PSUM") as psum,
        ):
            ident = consts.tile([P, P], F32)
            make_identity(nc, ident)
            ones_bf = consts.tile([P, 1], BF16)
            nc.vector.memset(ones_bf[:], 1.0)
            ones_row = consts.tile([1, P], F32)
            nc.vector.memset(ones_row[:], 1.0)
            bias_sb = consts.tile([P, HT], F32)
            nc.sync.dma_start(bias_sb[:], gb_d.ap().rearrange("(t p) -> p t", p=P))

            # ---- GWT [128hh, 8ht, 1024o] = G_weight.T, via PE transposes ----
            gwt = glob.tile([P, HT, H], F32)
            for ot in range(HT):
                gw_t = st.tile([P, H], F32, tag="ld")
                nc.sync.dma_start(gw_t[:], gw_a[ot])
                for g in range(2):
                    ps = psum.tile([P, 4, P], F32, tag="trps", bufs=2)
                    for j in range(4):
                        ht = 4 * g + j
                        nc.tensor.transpose(
                            ps[:, j, :], gw_t[:, ht * P:(ht + 1) * P], ident[:]
                        )
                    nc.vector.tensor_copy(
                        gwt[:, 4 * g:4 * (g + 1), ot * P:(ot + 1) * P], ps[:]
                    )

            for b in range(BPC):
                # ---- phase 1: qeT + qe_bf ----
                qeT = pb.tile([P, HT, LQ], F32, tag="qeT")
                qe_bf = pb.tile([P, QT, H], BF16, tag="qe_bf")
                for qt in range(QT):
                    qe_t = st.tile([P, H], F32, tag="ld")
                    nc.sync.dma_start(qe_t[:], qe_a[b, qt])
                    nc.vector.tensor_copy(qe_bf[:, qt, :], qe_t[:])
                    for g in range(2):
                        ps = psum.tile([P, 4, P], F32, tag="trps", bufs=2)
                        for j in range(4):
                            ht = 4 * g + j
                            nc.tensor.transpose(
                                ps[:, j, :], qe_t[:, ht * P:(ht + 1) * P], ident[:]
                            )
                        nc.vector.tensor_copy(
                            qeT[:, 4 * g:4 * (g + 1), qt * P:(qt + 1) * P], ps[:]
                        )

                # ---- phase 2: q_projT [128oo, 8ot, 512q] fp32 (+bias) ----
                qpT = pb.tile([P, HT, LQ], F32, tag="qpT")
                for ot in range(HT):
                    qp_ps = psum.tile([P, LQ], F32, tag="mmps", bufs=3)
                    for ht in range(HT):
                        nc.tensor.matmul(
                            qp_ps[:],
                            gwt[:, ht, ot * P:(ot + 1) * P],
                            qeT[:, ht, :],
                            start=(ht == 0),
                            stop=(ht == HT - 1),
                        )
                    nc.scalar.add(qpT[:, ot, :], qp_ps[:], bias_sb[:, ot:ot + 1])

                # ---- phases 3+4: stream p-tiles: transpose, att, E1, m_p ----
                att_sb = pb.tile([P, PT, LQ], F32, tag="att_sb")
                rmax_neg = sm.tile([P, PT], F32, tag="rmax")
                s1 = sm.tile([P, PT], F32, tag="s1")
                s1rec = sm.tile([P, PT], F32, tag="s1rec")
                m_part = sm.tile([P, LQ], F32, tag="M")
                for pt in range(PT):
                    pe_t = st.tile([P, H], F32, tag="ld")
                    nc.sync.dma_start(pe_t[:], pe_a[b, pt])
                    peT_t = st.tile([P, HT, P], F32, tag="peT", bufs=2)
                    for g in range(2):
                        ps = psum.tile([P, 4, P], F32, tag="trps", bufs=2)
                        for j in range(4):
                            ht = 4 * g + j
                            nc.tensor.transpose(
                                ps[:, j, :], pe_t[:, ht * P:(ht + 1) * P], ident[:]
                            )
                        nc.vector.tensor_copy(
                            peT_t[:, 4 * g:4 * (g + 1), :], ps[:]
                        )
                    att_ps = psum.tile([P, LQ], F32, tag="attps", bufs=2)
                    for ht in range(HT):
                        nc.tensor.matmul(
                            att_ps[:],
                            peT_t[:, ht, :],
                            qpT[:, ht, :],
                            start=(ht == 0),
                            stop=(ht == HT - 1),
                        )
                    nc.vector.reduce_max(
                        rmax_neg[:, pt:pt + 1], att_ps[:], axis=AX, negate=True
                    )
                    e1_t = st.tile([P, LQ], BF16, tag="e1")
                    nc.scalar.activation(
                        e1_t[:], att_ps[:], EXP,
                        bias=rmax_neg[:, pt:pt + 1], scale=1.0,
                        accum_out=s1[:, pt:pt + 1],
                    )
                    nc.vector.reciprocal(s1rec[:, pt:pt + 1], s1[:, pt:pt + 1])
                    nc.scalar.copy(att_sb[:, pt, :], att_ps[:])
                    if pt == 0:
                        nc.vector.tensor_copy(m_part[:], att_sb[:, pt, :])
                    else:
                        nc.vector.tensor_tensor(
                            m_part[:], m_part[:], att_sb[:, pt, :], MAX
                        )
                    # E1T tile [128qq, 4qt, 128pp] via bf16 DMA transpose
                    e1T_t = st.tile([P, QT, P], BF16, tag="e1T")
                    nc.sync.dma_start(e1T_t[:], e1_t[:], transpose=True)
                    # m_p for this p-tile
                    mp_sb = outp.tile([P, H], F32, tag="osb")
                    for nh in range(2):
                        mp_ps = psum.tile([P, 512], F32, tag="mmps", bufs=3)
                        for qt in range(QT):
                            nc.tensor.matmul(
                                mp_ps[:],
                                e1T_t[:, qt, :],
                                qe_bf[:, qt, nh * 512:(nh + 1) * 512],
                                start=(qt == 0),
                                stop=(qt == QT - 1),
                            )
                        nc.scalar.mul(
                            mp_sb[:, nh * 512:(nh + 1) * 512], mp_ps[:],
                            s1rec[:, pt:pt + 1],
                        )
                    nc.sync.dma_start(mp_a[b, pt], mp_sb[:])

                # ---- phase 5: column max -> cmax_row [1, 512] (negated) ----
                mt_ps = psum.tile([P, QT, P], F32, tag="trps", bufs=2)
                for qt in range(QT):
                    nc.tensor.transpose(
                        mt_ps[:, qt, :], m_part[:, qt * P:(qt + 1) * P], ident[:]
                    )
                cq_neg = sm.tile([P, QT], F32, tag="cq")
                nc.vector.reduce_max(cq_neg[:], mt_ps[:], axis=AX, negate=True)
                cqT_ps = psum.tile([P, P], F32, tag="trps", bufs=2)
                nc.tensor.transpose(cqT_ps[0:QT, :], cq_neg[:], ident[:])
                cqT_sb = sm.tile([QT, P], F32, tag="cqT")
                nc.vector.tensor_copy(cqT_sb[:], cqT_ps[0:QT, :])
                cmax_row = sm.tile([1, LQ], F32, tag="cmrow")
                nc.sync.dma_start(cmax_row[:], cqT_sb[:])
                # broadcast to [128, 512] via K=1 matmul, copy to SBUF
                cmb_ps = psum.tile([P, LQ], F32, tag="attps", bufs=2)
                nc.tensor.matmul(
                    cmb_ps[:], ones_row[:], cmax_row[:], start=True, stop=True
                )
                cmb = sm.tile([P, LQ], F32, tag="cmb")
                nc.vector.tensor_copy(cmb[:], cmb_ps[:])

                # ---- phase 6: E2 (bf16) + colsum via ones-matmul + pe_bf ----
                e2 = pb.tile([P, PT, LQ], BF16, tag="e2")
                pe_bf = pb.tile([P, PT, H], BF16, tag="pe_bf")
                c_ps = psum.tile([1, LQ], F32, tag="ones")
                for pt in range(PT):
                    nc.gpsimd.dma_start(pe_bf[:, pt, :], pe_a[b, pt])  # cast DMA
                    nc.vector.tensor_tensor(
                        att_sb[:, pt, :], att_sb[:, pt, :], cmb[:], ADD
                    )
                    nc.scalar.activation(e2[:, pt, :], att_sb[:, pt, :], EXP)
                    nc.tensor.matmul(
                        c_ps[:], ones_bf[:], e2[:, pt, :],
                        start=(pt == 0), stop=(pt == PT - 1),
                    )

                # ---- phase 7: crec [128, 4] ----
                c_row = sm.tile([1, LQ], F32, tag="crow")
                nc.vector.reciprocal(c_row[:], c_ps[:])
                crec = sm.tile([P, QT], F32, tag="crec")
                for t in range(QT):
                    nc.sync.dma_start(
                        crec[:, t:t + 1], c_row[0:1, t * P:(t + 1) * P]
                    )

                # ---- phase 8: m_q ----
                for qt in range(QT):
                    mq_sb = outp.tile([P, H], F32, tag="osb")
                    for nh in range(2):
                        mq_ps = psum.tile([P, 512], F32, tag="mmps", bufs=3)
                        for pt in range(PT):
                            nc.tensor.matmul(
                                mq_ps[:],
                                e2[:, pt, qt * P:(qt + 1) * P],
                                pe_bf[:, pt, nh * 512:(nh + 1) * 512],
                                start=(pt == 0),
                                stop=(pt == PT - 1),
                            )
                        nc.scalar.mul(
                            mq_sb[:, nh * 512:(nh + 1) * 512], mq_ps[:],
                            crec[:, qt:qt + 1],
                        )
                    nc.sync.dma_start(mq_a[b, qt], mq_sb[:])

    nc.finalize()
    return nc


_NC_CACHE = {}


def _get_nc():
    if "nc" not in _NC_CACHE:
        _NC_CACHE["nc"] = build_nc()
    return _NC_CACHE["nc"]


def kernel(p_encode, q_encode, G_weight, G_bias, trace=False):
    nc = _get_nc()
    in_maps = []
    for c in range(N_CORES):
        sl = slice(c * BPC, (c + 1) * BPC)
        in_maps.append(
            {
                "p_encode": np.ascontiguousarray(p_encode[sl]),
                "q_encode": np.ascontiguousarray(q_encode[sl]),
                "G_weight": np.ascontiguousarray(G_weight),
                "G_bias": np.ascontiguousarray(G_bias),
            }
        )
    res = run_bass_kernel_spmd(
        nc, in_maps, core_ids=list(range(N_CORES)), trace=trace
    )
    m_p = np.concatenate([r["m_p"] for r in res.results], axis=0)
    m_q = np.concatenate([r["m_q"] for r in res.results], axis=0)
    if trace:
        kernel.last_exec_time_ns = res.exec_time_ns
        kernel.last_results = res
    return (m_p, m_q)


# revision 6
# speedup vs baseline: 1.5110x; 1.5110x over previous
"""InterAttention kernel for Trainium2 (8 NeuronCores, data-parallel over batch).

Per batch b:
  q_proj = q_encode @ G_weight.T + G_bias            [LQ, H]
  att    = p_encode @ q_proj.T                       [LP, LQ]
  m_p    = softmax(att, axis=-1) @ q_encode          [LP, H]
  m_q    = softmax(att, axis=-2).T @ p_encode        [LQ, H]

B=32, LP=2048, LQ=512, H=1024.  8 cores x 4 batches each.
fp32 matmul chain through att (softmax-logit accuracy), bf16 post-softmax.
"""

import sys
import types

import numpy as np

# ---------------------------------------------------------------------------
# NTFF profile hook injection (lets run_bass_kernel_spmd(trace=True) work
# under axon). Harmless if the boot module is missing.
_HOOK = [None]


def _install_ntff_hook():
    if "antenv.axon_hooks" not in sys.modules:
        mod = types.ModuleType("antenv.axon_hooks")
        mod.set_axon_ntff_profile_hook = lambda h: _HOOK.__setitem__(0, h)
        mod.get_axon_ntff_profile_hook = lambda: _HOOK[0]
        sys.modules["antenv.axon_hooks"] = mod
        try:
            import antenv

            antenv.axon_hooks = mod
        except ImportError:
            pass
    mod = sys.modules["antenv.axon_hooks"]
    if mod.get_axon_ntff_profile_hook() is None:
        try:
            from trn_agent_boot.trn_boot import _ntff_profile_via_ctypes

            mod.set_axon_ntff_profile_hook(
                _ntff_profile_via_ctypes("/opt/axon/libaxon_pjrt.so")
            )
        except Exception:
            pass


_install_ntff_hook()

import concourse.bass as bass  # noqa: E402
import concourse.mybir as mybir  # noqa: E402
from concourse import bacc  # noqa: E402
from concourse.bass_utils import run_bass_kernel_spmd  # noqa: E402
from concourse.masks import make_identity  # noqa: E402
from concourse.tile import TileContext  # noqa: E402

P = 128
B, LP, LQ, H = 32, 2048, 512, 1024
N_CORES = 8
BPC = B // N_CORES  # batches per core
PT = LP // P  # 16 p-tiles
QT = LQ // P  # 4 q-tiles
HT = H // P  # 8 h-tiles
F32 = mybir.dt.float32
F32R = mybir.dt.float32r
import os
USE_F32R = os.environ.get('USE_F32R', '1') == '1'
BF16 = mybir.dt.bfloat16
EXP = mybir.ActivationFunctionType.Exp
ADD = mybir.AluOpType.add
MAX = mybir.AluOpType.max
AX = mybir.AxisListType.X


def build_nc(trace_scopes=False):
    nc = bacc.Bacc("TRN2", target_bir_lowering=False, debug=False)

    pe_d = nc.dram_tensor("p_encode", [BPC, LP, H], F32, kind="ExternalInput")
    qe_d = nc.dram_tensor("q_encode", [BPC, LQ, H], F32, kind="ExternalInput")
    gw_d = nc.dram_tensor("G_weight", [H, H], F32, kind="ExternalInput")
    gb_d = nc.dram_tensor("G_bias", [H], F32, kind="ExternalInput")
    mp_d = nc.dram_tensor("m_p", [BPC, LP, H], F32, kind="ExternalOutput")
    mq_d = nc.dram_tensor("m_q", [BPC, LQ, H], F32, kind="ExternalOutput")

    pe_a = pe_d.ap().rearrange("b (t p) h -> b t p h", p=P)  # [BPC,16,128,1024]
    qe_a = qe_d.ap().rearrange("b (t p) h -> b t p h", p=P)  # [BPC,4,128,1024]
    gw_a = gw_d.ap().rearrange("(t p) h -> t p h", p=P)  # [8,128,1024]
    mp_a = mp_d.ap().rearrange("b (t p) h -> b t p h", p=P)
    mq_a = mq_d.ap().rearrange("b (t p) h -> b t p h", p=P)

    with TileContext(nc) as tc:
        with (
            tc.tile_pool(name="consts", bufs=1) as consts,
            tc.tile_pool(name="glob", bufs=1) as glob,
            tc.tile_pool(name="per_batch", bufs=1) as pb,
            tc.tile_pool(name="stream", bufs=2) as st,
            tc.tile_pool(name="small", bufs=2) as sm,
            tc.tile_pool(name="outs", bufs=2) as outp,
            tc.tile_pool(name="psum", bufs=1, space="PSUM") as psum,
        ):
            ident = consts.tile([P, P], F32)
            make_identity(nc, ident)
            ones_bf = consts.tile([P, 1], BF16)
            nc.vector.memset(ones_bf[:], 1.0)
            ones_row = consts.tile([1, P], F32)
            nc.vector.memset(ones_row[:], 1.0)
            bias_sb = consts.tile([P, HT], F32)
            nc.sync.dma_start(bias_sb[:], gb_d.ap().rearrange("(t p) -> p t", p=P))

            # ---- GWT [128hh, 8ht, 1024o] = G_weight.T, via PE transposes ----
            AF = F32R if USE_F32R else F32
            gwt = glob.tile([P, HT, H], AF)
            for ot in range(HT):
                gw_t = st.tile([P, H], F32, tag="ld")
                nc.sync.dma_start(gw_t[:], gw_a[ot])
                for g in range(2):
                    ps = psum.tile([P, 4, P], F32, tag="trps", bufs=2)
                    for j in range(4):
                        ht = 4 * g + j
                        nc.tensor.transpose(
                            ps[:, j, :], gw_t[:, ht * P:(ht + 1) * P], ident[:]
                        )
                    nc.vector.tensor_copy(
                        gwt[:, 4 * g:4 * (g + 1), ot * P:(ot + 1) * P], ps[:]
                    )

            for b in range(BPC):
                # ---- phase 1: qeT + qe_bf ----
                qeT = pb.tile([P, HT, LQ], AF, tag="qeT")
                qe_bf = pb.tile([P, QT, H], BF16, tag="qe_bf")
                for qt in range(QT):
                    qe_t = st.tile([P, H], F32, tag="ld")
                    nc.sync.dma_start(qe_t[:], qe_a[b, qt])
                    nc.vector.tensor_copy(qe_bf[:, qt, :], qe_t[:])
                    for g in range(2):
                        ps = psum.tile([P, 4, P], F32, tag="trps", bufs=2)
                        for j in range(4):
                            ht = 4 * g + j
                            nc.tensor.transpose(
                                ps[:, j, :], qe_t[:, ht * P:(ht + 1) * P], ident[:]
                            )
                        nc.vector.tensor_copy(
                            qeT[:, 4 * g:4 * (g + 1), qt * P:(qt + 1) * P], ps[:]
                        )

                # ---- phase 2: q_projT [128oo, 8ot, 512q] fp32 (+bias) ----
                qpT = pb.tile([P, HT, LQ], AF, tag="qpT")
                for ot in range(HT):
                    qp_ps = psum.tile([P, LQ], F32, tag="mmps", bufs=3)
                    for ht in range(HT):
                        nc.tensor.matmul(
                            qp_ps[:],
                            gwt[:, ht, ot * P:(ot + 1) * P],
                            qeT[:, ht, :],
                            start=(ht == 0), stop=(ht == HT - 1),
                        )
                    nc.scalar.add(qpT[:, ot, :], qp_ps[:], bias_sb[:, ot:ot + 1])

                # ---- phases 3+4: stream p-tiles: transpose, att, E1, m_p ----
                att_tiles = []
                rmax_neg = sm.tile([P, PT], F32, tag="rmax")
                s1 = sm.tile([P, PT], F32, tag="s1")
                s1rec = sm.tile([P, PT], F32, tag="s1rec")
                m_part = sm.tile([P, LQ], F32, tag="M")
                for pt in range(PT):
                    pe_t = st.tile([P, H], F32, tag="ld")
                    nc.sync.dma_start(pe_t[:], pe_a[b, pt])
                    peT_t = st.tile([P, HT, P], AF, tag="peT", bufs=2)
                    for g in range(2):
                        ps = psum.tile([P, 4, P], F32, tag="trps", bufs=2)
                        for j in range(4):
                            ht = 4 * g + j
                            nc.tensor.transpose(
                                ps[:, j, :], pe_t[:, ht * P:(ht + 1) * P], ident[:]
                            )
                        nc.vector.tensor_copy(
                            peT_t[:, 4 * g:4 * (g + 1), :], ps[:]
                        )
                    att_ps = psum.tile([P, LQ], F32, tag="attps", bufs=2)
                    for ht in range(HT):
                        nc.tensor.matmul(
                            att_ps[:],
                            peT_t[:, ht, :],
                            qpT[:, ht, :],
                            start=(ht == 0), stop=(ht == HT - 1),
                        )
                    nc.vector.reduce_max(
                        rmax_neg[:, pt:pt + 1], att_ps[:], axis=AX, negate=True
                    )
                    e1_t = st.tile([P, LQ], BF16, tag="e1")
                    nc.scalar.activation(
                        e1_t[:], att_ps[:], EXP,
                        bias=rmax_neg[:, pt:pt + 1], scale=1.0,
                        accum_out=s1[:, pt:pt + 1],
                    )
                    nc.vector.reciprocal(s1rec[:, pt:pt + 1], s1[:, pt:pt + 1])
                    att_t = st.tile([P, LQ], F32, tag="attsb", bufs=18)
                    att_tiles.append(att_t)
                    nc.scalar.copy(att_t[:], att_ps[:])
                    if pt == 0:
                        nc.vector.tensor_copy(m_part[:], att_t[:])
                    else:
                        nc.vector.tensor_tensor(
                            m_part[:], m_part[:], att_t[:], MAX
                        )
                    # E1T tile [128qq, 4qt, 128pp] via bf16 DMA transpose
                    e1T_t = st.tile([P, QT, P], BF16, tag="e1T")
                    nc.sync.dma_start(e1T_t[:], e1_t[:], transpose=True)
                    # m_p for this p-tile
                    mp_sb = outp.tile([P, H], F32, tag="osb")
                    for nh in range(2):
                        mp_ps = psum.tile([P, 512], F32, tag="mmps", bufs=3)
                        for qt in range(QT):
                            nc.tensor.matmul(
                                mp_ps[:],
                                e1T_t[:, qt, :],
                                qe_bf[:, qt, nh * 512:(nh + 1) * 512],
                                start=(qt == 0),
                                stop=(qt == QT - 1),
                            )
                        nc.scalar.mul(
                            mp_sb[:, nh * 512:(nh + 1) * 512], mp_ps[:],
                            s1rec[:, pt:pt + 1],
                        )
                    nc.sync.dma_start(mp_a[b, pt], mp_sb[:])

                # ---- phase 5: column max -> cmax_row [1, 512] (negated) ----
                mt_ps = psum.tile([P, QT, P], F32, tag="trps", bufs=2)
                for qt in range(QT):
                    nc.tensor.transpose(
                        mt_ps[:, qt, :], m_part[:, qt * P:(qt + 1) * P], ident[:]
                    )
                cq_neg = sm.tile([P, QT], F32, tag="cq")
                nc.vector.reduce_max(cq_neg[:], mt_ps[:], axis=AX, negate=True)
                cqT_ps = psum.tile([P, P], F32, tag="trps", bufs=2)
                nc.tensor.transpose(cqT_ps[0:QT, :], cq_neg[:], ident[:])
                cqT_sb = sm.tile([QT, P], F32, tag="cqT")
                nc.vector.tensor_copy(cqT_sb[:], cqT_ps[0:QT, :])
                cmax_row = sm.tile([1, LQ], F32, tag="cmrow")
                nc.sync.dma_start(cmax_row[:], cqT_sb[:])
                # broadcast to [128, 512] via K=1 matmul, copy to SBUF
                cmb_ps = psum.tile([P, LQ], F32, tag="attps", bufs=2)
                nc.tensor.matmul(
                    cmb_ps[:], ones_row[:], cmax_row[:], start=True, stop=True
                )
                cmb = sm.tile([P, LQ], F32, tag="cmb")
                nc.vector.tensor_copy(cmb[:], cmb_ps[:])

                # ---- phase 6: E2 (bf16) + colsum via ones-matmul + pe_bf ----
                e2 = pb.tile([P, PT, LQ], BF16, tag="e2")
                pe_bf = pb.tile([P, PT, H], BF16, tag="pe_bf")
                c_ps = psum.tile([1, LQ], F32, tag="ones")
                for pt in range(PT):
                    nc.gpsimd.dma_start(pe_bf[:, pt, :], pe_a[b, pt])  # cast DMA
                    att_t = att_tiles[pt]
                    nc.vector.tensor_tensor(att_t[:], att_t[:], cmb[:], ADD)
                    nc.scalar.activation(e2[:, pt, :], att_t[:], EXP)
                    nc.tensor.matmul(
                        c_ps[:], ones_bf[:], e2[:, pt, :],
                        start=(pt == 0), stop=(pt == PT - 1),
                    )

                # ---- phase 7: crec [128, 4] ----
                c_row = sm.tile([1, LQ], F32, tag="crow")
                nc.vector.reciprocal(c_row[:], c_ps[:])
                crec = sm.tile([P, QT], F32, tag="crec")
                for t in range(QT):
                    nc.sync.dma_start(
                        crec[:, t:t + 1], c_row[0:1, t * P:(t + 1) * P]
                    )

                # ---- phase 8: m_q ----
                for qt in range(QT):
                    mq_sb = outp.tile([P, H], F32, tag="osb")
                    for nh in range(2):
                        mq_ps = psum.tile([P, 512], F32, tag="mmps", bufs=3)
                        for pt in range(PT):
                            nc.tensor.matmul(
                                mq_ps[:],
                                e2[:, pt, qt * P:(qt + 1) * P],
                                pe_bf[:, pt, nh * 512:(nh + 1) * 512],
                                start=(pt == 0),
                                stop=(pt == PT - 1),
                            )
                        nc.scalar.mul(
                            mq_sb[:, nh * 512:(nh + 1) * 512], mq_ps[:],
                            crec[:, qt:qt + 1],
                        )
                    nc.sync.dma_start(mq_a[b, qt], mq_sb[:])

    nc.finalize()
    return nc


_NC_CACHE = {}


def _get_nc():
    if "nc" not in _NC_CACHE:
        _NC_CACHE["nc"] = build_nc()
    return _NC_CACHE["nc"]


def kernel(p_encode, q_encode, G_weight, G_bias, trace=False):
    nc = _get_nc()
    in_maps = []
    for c in range(N_CORES):
        sl = slice(c * BPC, (c + 1) * BPC)
        in_maps.append(
            {
                "p_encode": np.ascontiguousarray(p_encode[sl]),
                "q_encode": np.ascontiguousarray(q_encode[sl]),
                "G_weight": np.ascontiguousarray(G_weight),
                "G_bias": np.ascontiguousarray(G_bias),
            }
        )
    res = run_bass_kernel_spmd(
        nc, in_maps, core_ids=list(range(N_CORES)), trace=trace
    )
    m_p = np.concatenate([r["m_p"] for r in res.results], axis=0)
    m_q = np.concatenate([r["m_q"] for r in res.results], axis=0)
    if trace:
        kernel.last_exec_time_ns = res.exec_time_ns
        kernel.last_results = res
    return (m_p, m_q)
